# revision 17
# baseline (speedup 1.0000x reference)
"""Trainium2 Bass kernel for the pairwise-score attention + gated MLP encoding.

Computation (per batch element b, p=1024 tokens, d=256 features):
    A[i,j]  = wa.P_i + wb.P_j + (P_i*wc).P_j
    itr     = softmax_j(A) @ P
    cat     = [P, itr]
    z       = tanh(cat@w1+b1); r = sigmoid(cat@w2+b2); f = sigmoid(cat@w3+b3)
    out     = r*P + f*z

Sharding: data-parallel over batch across 8 NeuronCores (4 batch el / core).

Kernel structure per batch element (everything stays on-chip):
  - P^T built from natural-layout P via PE transposes.
  - Scores computed transposed: S^T[j,i] = sum_d PT[d,j]*PcT[d,i] (fp32r
    matmuls, K=256 accumulated in PSUM).  The wa.P_i term is constant along
    the softmax axis j and cancels -> never computed.  The wb.P_j term is
    per-partition in this layout -> folded into the exp as an ACT bias.
  - exp on the scalar engine straight out of PSUM (scores are O(+-4), no
    max-subtraction needed).
  - Attention numerator AND softmax denominator in one matmul chain:
    lhsT=expS^T, rhs=[P | 1] -> psum[i, 0:256]=numerator, psum[i,256]=denom.
  - MLP computed transposed (out^T = (cat@w)^T) so b1/b2/b3 are per-partition
    ACT biases; sigmoid evaluated as 0.5+0.5*tanh(x/2) to keep every
    activation in the one "exp_and_others" ACT table set (no table reloads).
  - Final gating on DVE against P^T, PE-transposed back and stored
    contiguously.
"""

import os
import sys

if "/opt/trn_rl_repo" not in sys.path:
    sys.path.insert(0, "/opt/trn_rl_repo")

import numpy as np

import concourse.bass as bass
import concourse.mybir as mybir
import concourse.tile as tile
from concourse import bacc
from concourse.bass_utils import run_bass_kernel_spmd
from concourse.masks import make_identity

F32 = mybir.dt.float32
F32R = mybir.dt.float32r
AF = mybir.ActivationFunctionType
ALU = mybir.AluOpType

B, PLEN, D = 32, 1024, 256
N_CORES = 8
B_LOC = B // N_CORES  # batch elements per core

NJ = PLEN // 128  # 8 token chunks of 128
ND = D // 128     # 2 feature chunks of 128


def _r(ap):
    """fp32 -> fp32r bitcast at matmul callsites (full-rate PE at N>=256)."""
    return ap.bitcast(F32R)


def _emit(ctx, tc, P_in, w_att, w_mlp, b_mlp, out):
    nc = tc.nc

    const = ctx.enter_context(tc.tile_pool(name="const", bufs=1))
    pin = ctx.enter_context(tc.tile_pool(name="pin", bufs=2))
    ptp = ctx.enter_context(tc.tile_pool(name="ptp", bufs=2))
    pexp = ctx.enter_context(tc.tile_pool(name="pexp", bufs=1))
    pitr = ctx.enter_context(tc.tile_pool(name="pitr", bufs=2))
    pmlp = ctx.enter_context(tc.tile_pool(name="pmlp", bufs=2))
    pout = ctx.enter_context(tc.tile_pool(name="pout", bufs=2))
    ps_big = ctx.enter_context(tc.tile_pool(name="ps_big", bufs=3, space="PSUM"))
    ps_att = ctx.enter_context(tc.tile_pool(name="ps_att", bufs=2, space="PSUM"))
    ps_t = ctx.enter_context(tc.tile_pool(name="ps_t", bufs=3, space="PSUM"))

    # ---- constants (once per core) ----
    ident = const.tile([128, 128], F32)
    make_identity(nc, ident)

    # w_itr_att = [wa (unused) ; wb ; wc], each d=256 -> two 128-chunks
    wc_sb = []
    for dc in range(ND):
        wc = const.tile([128, 1], F32, tag=f"wc{dc}")
        nc.gpsimd.dma_start(out=wc, in_=w_att[bass.ds(2 * D + dc * 128, 128)].unsqueeze(1))
        wc_sb.append(wc)
    # wb broadcast to all partitions: [128, 256] (for the DVE sb reduction)
    wbb = const.tile([128, D], F32)
    _wbs = w_att[bass.ds(D, D)]
    nc.gpsimd.dma_start(
        out=wbb,
        in_=bass.AP(tensor=_wbs.tensor, offset=_wbs.offset, ap=[[0, 128]] + list(_wbs.ap)),
    )

    # MLP weights: [512, 256] -> sbuf [128, 4(kc), 256]
    w_sb = []
    for wi in range(3):
        wt = const.tile([128, 4, D], F32R, tag=f"w{wi}")
        nc.gpsimd.dma_start(
            out=wt, in_=w_mlp[wi].rearrange("(kc k) d -> k kc d", k=128).bitcast(F32R))
        w_sb.append(wt)

    # biases, per dout-chunk [128,1]; for r/f (sigmoid-via-tanh) we need b/2
    b_sb = []  # b_sb[wi][dc]
    for wi in range(3):
        chunks = []
        for dc in range(ND):
            bt = const.tile([128, 1], F32, tag=f"b{wi}{dc}")
            nc.gpsimd.dma_start(out=bt, in_=b_mlp[wi][bass.ds(dc * 128, 128)].unsqueeze(1))
            if wi > 0:
                bh = const.tile([128, 1], F32, tag=f"bh{wi}{dc}")
                nc.scalar.mul(out=bh, in_=bt, mul=0.5)
                bt = bh
            chunks.append(bt)
        b_sb.append(chunks)

    # ---- per batch element ----
    for b in range(B_LOC):
        # 1. load P natural layout, augmented with a ones column (for the
        #    softmax-denominator trick in the attention matmul)
        Pn = []
        for jc in range(NJ):
            t = pin.tile([128, D + 4], F32, tag=f"pn{jc}")
            nc.sync.dma_start(out=t[:, 0:D], in_=P_in[b, bass.ts(jc, 128), :])
            nc.vector.memset(t[:, D : D + 4], 1.0)
            Pn.append(t)
        Pn_r = []
        for jc in range(NJ):
            tr = pin.tile([128, D + 4], F32R, tag=f"pnr{jc}", name=f"pnr{jc}")
            nc.vector.tensor_copy(out=tr, in_=Pn[jc])
            Pn_r.append(tr)

        if int(os.environ.get("KPHASE", "9")) == 0:
            for jc in range(NJ):
                nc.sync.dma_start(out=out[b, bass.ts(jc, 128), :],
                                  in_=Pn_r[jc][:, 0:D].bitcast(F32))
            continue

        if int(os.environ.get("KPHASE", "9")) == 6:
            for jc in range(NJ):
                scr = pin.tile([128, D], F32, tag="sbscr")
                nc.vector.tensor_mul(out=scr, in0=Pn[jc][:, 0:D], in1=wbb)
                nc.sync.dma_start(out=out[b, bass.ts(jc, 128), :], in_=scr)
            continue

        if int(os.environ.get("KPHASE", "9")) == 5:
            sb_sb = []
            for jc in range(NJ):
                scr = pin.tile([128, D], F32, tag="sbscr")
                s = pin.tile([128, 1], F32, tag=f"sbj{jc}")
                nc.vector.tensor_tensor_reduce(
                    out=scr, in0=Pn[jc][:, 0:D], in1=wbb, scale=1.0, scalar=0.0,
                    op0=ALU.mult, op1=ALU.add, accum_out=s)
                sb_sb.append(s)
                nc.vector.tensor_scalar_add(out=scr[:, 0:1], in0=scr[:, 0:1], scalar1=s)
                nc.sync.dma_start(out=out[b, bass.ts(jc, 128), :], in_=scr)
            continue

        # 2. P^T via PE transposes: PT[dc] is [128(d), 1024(j)]
        PT = [ptp.tile([128, PLEN], F32R, tag=f"pt{dc}", name=f"PT{dc}") for dc in range(ND)]
        for jc in range(NJ):
            for dc in range(ND):
                pst = ps_t.tile([128, 128], F32, tag="pst")
                nc.tensor.transpose(pst, Pn[jc][:, bass.ts(dc, 128)], ident)
                nc.vector.tensor_copy(out=PT[dc][:, bass.ts(jc, 128)], in_=pst)

        # 3. PcT = PT * wc (per-partition scalar on d)
        PcT = [ptp.tile([128, PLEN], F32R, tag=f"pct{dc}", name=f"PcT{dc}") for dc in range(ND)]
        for dc in range(ND):
            nc.vector.tensor_scalar_mul(out=PcT[dc], in0=PT[dc].bitcast(F32),
                                        scalar1=wc_sb[dc])

        # 4. sb[j] = P_j . wb on DVE: row-reduce of Pn * wb_broadcast
        sb_sb = []
        for jc in range(NJ):
            scr = pin.tile([128, D], F32, tag="sbscr")
            s = pin.tile([128, 1], F32, tag=f"sbj{jc}")
            nc.vector.tensor_mul(out=scr, in0=Pn[jc][:, 0:D], in1=wbb)
            nc.vector.reduce_sum(out=s, in_=scr, axis=mybir.AxisListType.X)
            sb_sb.append(s)

        if int(os.environ.get("KPHASE", "9")) == 1:
            for jc in range(NJ):
                nc.sync.dma_start(out=out[b, bass.ts(jc, 128), :],
                                  in_=PT[jc % 2][:, bass.ts(jc // 2, 256)].bitcast(F32))
                nc.vector.tensor_scalar_mul(out=PcT[0][:, 0:1], in0=sb_sb[jc],
                                            scalar1=wc_sb[0])
            continue

        # 5. scores + exp: expST[jc][j, i] = exp(sb[j] + sum_d P[j]wc P[i])
        expST = [pexp.tile([128, PLEN], F32R, tag=f"es{jc}", name=f"expST{jc}") for jc in range(NJ)]
        for jc in range(NJ):
            for ic2 in range(2):
                pss = ps_big.tile([128, 512], F32, tag="big")
                nc.tensor.matmul(pss, PT[0][:, bass.ts(jc, 128)],
                                 PcT[0][:, bass.ts(ic2, 512)],
                                 start=True, stop=False)
                nc.tensor.matmul(pss, PT[1][:, bass.ts(jc, 128)],
                                 PcT[1][:, bass.ts(ic2, 512)],
                                 start=False, stop=True)
                nc.scalar.activation(out=expST[jc][:, bass.ts(ic2, 512)], in_=pss,
                                     func=AF.Exp, bias=sb_sb[jc], scale=1.0)

        if int(os.environ.get("KPHASE", "9")) == 2:
            for jc in range(NJ):
                nc.sync.dma_start(out=out[b, bass.ts(jc, 128), :],
                                  in_=expST[jc][:, 0:D].bitcast(F32))
            continue

        # 6. attention numerator + denominator, then normalize and transpose
        itrT = [pitr.tile([128, PLEN], F32R, tag=f"it{dc}", name=f"itrT{dc}") for dc in range(ND)]
        for ic in range(NJ):
            psa = ps_att.tile([128, D + 4], F32, tag="att")
            for jc in range(NJ):
                nc.tensor.matmul(psa, expST[jc][:, bass.ts(ic, 128)], Pn_r[jc],
                                 start=(jc == 0), stop=(jc == NJ - 1))
            recip = pitr.tile([128, 1], F32, tag="recip")
            nc.vector.reciprocal(out=recip, in_=psa[:, D : D + 1])
            itr_n = pitr.tile([128, D], F32, tag="itrn")
            nc.vector.tensor_scalar_mul(out=itr_n, in0=psa[:, 0:D], scalar1=recip)
            for dc in range(ND):
                pst = ps_t.tile([128, 128], F32, tag="pst")
                nc.tensor.transpose(pst, itr_n[:, bass.ts(dc, 128)], ident)
                nc.vector.tensor_copy(out=itrT[dc][:, bass.ts(ic, 128)], in_=pst)

        if int(os.environ.get("KPHASE", "9")) == 3:
            for jc in range(NJ):
                nc.sync.dma_start(out=out[b, bass.ts(jc, 128), :],
                                  in_=itrT[jc % 2][:, bass.ts(jc // 2, 256)].bitcast(F32))
            continue

        catT = [PT[0], PT[1], itrT[0], itrT[1]]

        # 7. MLP (transposed) + gating + output transpose
        onat = [pout.tile([128, D], F32, tag=f"on{p2}", name=f"onat{p2}") for p2 in range(NJ)]
        for pc in range(2):
            for dc in range(ND):
                acts = []
                for wi in range(3):
                    psm = ps_big.tile([128, 512], F32, tag="big")
                    for kc in range(4):
                        nc.tensor.matmul(
                            psm,
                            w_sb[wi][:, kc, bass.ts(dc, 128)],
                            catT[kc][:, bass.ts(pc, 512)],
                            start=(kc == 0), stop=(kc == 3),
                        )
                    t = pmlp.tile([128, 512], F32, tag=f"act{wi}")
                    if wi == 0:
                        # z = tanh(u + b1)
                        nc.scalar.activation(out=t, in_=psm, func=AF.Tanh,
                                             bias=b_sb[0][dc], scale=1.0)
                    else:
                        # sigmoid(u + b) = 0.5 + 0.5*tanh(u/2 + b/2)
                        nc.scalar.activation(out=t, in_=psm, func=AF.Tanh,
                                             bias=b_sb[wi][dc], scale=0.5)
                        nc.vector.tensor_scalar(out=t, in0=t, scalar1=0.5,
                                                scalar2=0.5, op0=ALU.mult,
                                                op1=ALU.add)
                    acts.append(t)
                z_t, r_t, f_t = acts
                m1 = pmlp.tile([128, 512], F32, tag="m1")
                nc.vector.tensor_mul(out=m1, in0=r_t,
                                     in1=PT[dc][:, bass.ts(pc, 512)].bitcast(F32))
                m2 = pmlp.tile([128, 512], F32, tag="m2")
                nc.vector.tensor_mul(out=m2, in0=f_t, in1=z_t)
                oT = pmlp.tile([128, 512], F32, tag="oT")
                nc.vector.tensor_add(out=oT, in0=m1, in1=m2)
                for blk in range(4):
                    p2 = pc * 4 + blk
                    pst = ps_t.tile([128, 128], F32, tag="pst")
                    nc.tensor.transpose(pst, oT[:, bass.ts(blk, 128)], ident)
                    nc.vector.tensor_copy(out=onat[p2][:, bass.ts(dc, 128)], in_=pst)

        for p2 in range(NJ):
            nc.sync.dma_start(out=out[b, bass.ts(p2, 128), :], in_=onat[p2])


_NC_CACHE = {}


def _build():
    if "nc" in _NC_CACHE:
        return _NC_CACHE["nc"]
    nc = bacc.Bacc("TRN2", target_bir_lowering=False, debug=False,
                   num_devices=N_CORES)
    P_in = nc.dram_tensor("p_in", [B_LOC, PLEN, D], F32, kind="ExternalInput").ap()
    w_att = nc.dram_tensor("w_att", [3 * D], F32, kind="ExternalInput").ap()
    w_mlp = [nc.dram_tensor(f"w{i}", [2 * D, D], F32, kind="ExternalInput").ap()
             for i in (1, 2, 3)]
    b_mlp = [nc.dram_tensor(f"b{i}", [D], F32, kind="ExternalInput").ap()
             for i in (1, 2, 3)]
    out = nc.dram_tensor("out", [B_LOC, PLEN, D], F32, kind="ExternalOutput").ap()

    from contextlib import ExitStack

    with tile.TileContext(nc) as tc, ExitStack() as ctx:
        _emit(ctx, tc, P_in, w_att, w_mlp, b_mlp, out)
    nc.compile()
    _NC_CACHE["nc"] = nc
    return nc


def run(inputs, trace=False, tmpdir=None):
    nc = _build()
    P = np.ascontiguousarray(np.asarray(inputs["P"], dtype=np.float32))
    shared = {
        "w_att": np.ascontiguousarray(np.asarray(inputs["w_itr_att"], np.float32)),
        "w1": np.ascontiguousarray(np.asarray(inputs["w1"], np.float32)),
        "w2": np.ascontiguousarray(np.asarray(inputs["w2"], np.float32)),
        "w3": np.ascontiguousarray(np.asarray(inputs["w3"], np.float32)),
        "b1": np.ascontiguousarray(np.asarray(inputs["b1"], np.float32)),
        "b2": np.ascontiguousarray(np.asarray(inputs["b2"], np.float32)),
        "b3": np.ascontiguousarray(np.asarray(inputs["b3"], np.float32)),
    }
    in_maps = [
        {"p_in": P[c * B_LOC : (c + 1) * B_LOC], **shared} for c in range(N_CORES)
    ]
    res = run_bass_kernel_spmd(nc, in_maps, list(range(N_CORES)), trace=trace,
                               tmpdir=tmpdir)
    full = np.concatenate([res.results[c]["out"] for c in range(N_CORES)], axis=0)
    return full, res


def kernel(**inputs):
    full, _ = run(inputs)
    return full
